# revision 15
# baseline (speedup 1.0000x reference)
"""BlockGRUCell Trainium2 kernel.

Computation (per reference):
  hx = concat([h, x], -1)                       # (B, 2048)
  gate[b, 192g+o] = sum_i hx[b, 128g+i] * W[g, o, i]   # block-diagonal matmul
  r, c, u = split(gate + bias, 3)               # bias == 0 from setup_inputs
  h_new = sigmoid(u) * tanh(sigmoid(r) * c) + (1 - sigmoid(u)) * h

Sharding: data-parallel over batch across 8 NeuronCores (2048 rows each),
weights replicated.

The TensorE matmul contracts over the partition dim, so the stationary
operand must be hx^T per 128-feature block. The host pre-packs x and h into
per-tile transposed bf16 panels (doing this on device costs a PE transpose
plus a PSUM->SBUF cast that saturates VectorE/ScalarE). Work is organized in
pairs of 128-row tiles so DMAs move 1 MiB chunks and the SBUF-only
elementwise ops run on [128, 2048] panels (halved instruction overhead).

Per core, per 128-row tile:
  - 20 block matmuls (bf16, fp32 accum) into three [128, 1024] PSUM panels
    (= r/c/u exactly; matmuls split at PSUM bank/panel crossings)
  - ScalarE: sigmoid(r), sigmoid(u);  VectorE: rc = reset*c from PSUM
Per pair of tiles:
  - ScalarE: cand = tanh(rc) on [128, 2048]
  - VectorE: h_new = h + upd*(cand - h) as three [128, 2048] ops
    (fp32 tensor_tensor is 1x everywhere; GpSimd would steal DVE's port)
"""

import numpy as np
import ml_dtypes

import concourse.bass as bass
import concourse.bacc as bacc
import concourse.tile as tile
import concourse.mybir as mybir
from concourse.bass_utils import run_bass_kernel_spmd

N_CORES = 8
BATCH = 16384
BS = BATCH // N_CORES            # rows per core
P = 128
NT = BS // P                     # 128-row tiles per core
NP = NT // 2                     # tile pairs per core
HID = 1024
G = 16                           # feature blocks
IN_PER = 128
OUT_PER = 192
GATE = 3 * HID                   # 3072
PSUM_BANK_F32 = 512
H2 = 2 * HID

F32 = mybir.dt.float32
BF16 = mybir.dt.bfloat16
AFT = mybir.ActivationFunctionType


def _body(tc, nc, hxt_d, h_d, wt_d, out_d):
    with (
        tc.tile_pool(name="consts", bufs=1) as consts,
        tc.tile_pool(name="io", bufs=3) as io,
        tc.tile_pool(name="resetp", bufs=4) as resetp,
        tc.tile_pool(name="panels", bufs=2) as panels,
        tc.tile_pool(name="gatep", bufs=4, space="PSUM") as gatep,
    ):
        wt_s = consts.tile([P, G * OUT_PER], BF16)
        nc.sync.dma_start(out=wt_s, in_=wt_d[:, :])

        for q in range(NP):
            hxt = io.tile([P, 2 * H2], BF16, tag="hxt")
            h2 = io.tile([P, H2], F32, tag="h2")
            nc.sync.dma_start(out=hxt, in_=hxt_d[q])
            nc.sync.dma_start(out=h2, in_=h_d[q])

            rc2 = panels.tile([P, H2], F32, tag="rc2")
            upd2 = panels.tile([P, H2], F32, tag="upd2")

            for s in range(2):
                # gate panels = the r/c/u split exactly (2 PSUM banks each)
                gR = gatep.tile([P, HID], F32, tag="gate")
                gC = gatep.tile([P, HID], F32, tag="gate")
                gU = gatep.tile([P, HID], F32, tag="gate")
                gs = (gR, gC, gU)

                for g in range(G):
                    lhsT = hxt[:, s * H2 + g * P:s * H2 + (g + 1) * P]
                    w0 = g * OUT_PER
                    # split matmul writes at PSUM bank (512) boundaries
                    c0 = w0
                    while c0 < w0 + OUT_PER:
                        c1 = min(w0 + OUT_PER,
                                 (c0 // PSUM_BANK_F32 + 1) * PSUM_BANK_F32)
                        gate = gs[c0 // HID]
                        nc.tensor.matmul(
                            gate[:, c0 % HID:(c0 % HID) + c1 - c0],
                            lhsT, wt_s[:, c0:c1], start=True, stop=True)
                        c0 = c1

                reset = resetp.tile([P, HID], F32, tag="reset")
                nc.scalar.activation(reset, gR, AFT.Sigmoid)
                nc.vector.tensor_tensor(rc2[:, s * HID:(s + 1) * HID],
                                        gC, reset, mybir.AluOpType.mult)
                nc.scalar.activation(upd2[:, s * HID:(s + 1) * HID],
                                     gU, AFT.Sigmoid)

            cand2 = panels.tile([P, H2], F32, tag="cand2")
            nc.scalar.activation(cand2, rc2, AFT.Tanh)
            # h_new = h + upd*(cand - h)
            dd2 = panels.tile([P, H2], F32, tag="dd2")
            nc.vector.tensor_sub(dd2, cand2, h2)
            ee2 = panels.tile([P, H2], F32, tag="ee2")
            nc.vector.tensor_mul(ee2, upd2, dd2)
            hn2 = panels.tile([P, H2], F32, tag="hn2")
            nc.vector.tensor_add(hn2, h2, ee2)
            nc.sync.dma_start(out=out_d[q], in_=hn2)


_NC_CACHE = {}


def _build_nc():
    if "nc" in _NC_CACHE:
        return _NC_CACHE["nc"]
    nc = bacc.Bacc()
    hxt_d = nc.dram_tensor("hxt", [NP, P, 2 * H2], BF16, kind="ExternalInput")
    h_d = nc.dram_tensor("h2", [NP, P, H2], F32, kind="ExternalInput")
    wt_d = nc.dram_tensor("wt", [P, G * OUT_PER], BF16, kind="ExternalInput")
    out_d = nc.dram_tensor("out", [NP, P, H2], F32, kind="ExternalOutput")
    with tile.TileContext(nc) as tc:
        _body(tc, nc, hxt_d, h_d, wt_d, out_d)
    nc.compile()
    _NC_CACHE["nc"] = nc
    return nc


def _np_reference(x, h, weight, bias):
    hx = np.concatenate([h, x], axis=-1)
    xg = hx.reshape(x.shape[0], G, IN_PER)
    gate = np.einsum("bgi,goi->bgo", xg, weight).reshape(x.shape[0], GATE)
    gate = gate + bias
    r, c, u = np.split(gate, 3, axis=-1)
    reset = 1.0 / (1.0 + np.exp(-r))
    cand = np.tanh(reset * c)
    upd = 1.0 / (1.0 + np.exp(-u))
    return (upd * cand + (1.0 - upd) * h).astype(np.float32)


def _pack_hxt(hs, xs):
    """-> [NP, 128, 4096] bf16; cols [s*2048 + 1024g + p ...]:
    hxt[q, p, s*2048 + 0:1024]   = h^T tile 2q+s  (feature-major blocks)
    hxt[q, p, s*2048 + 1024:2048] = x^T tile 2q+s."""
    def tp(a):                      # [BS, 1024] -> [NT, 128, 1024] transposed
        t = a.reshape(NT, P, 8, P).transpose(0, 3, 2, 1)    # [t, p, g, b]
        return t.reshape(NT, P, 8 * P)
    ht, xt = tp(hs), tp(xs)
    arr = np.concatenate([ht, xt], axis=2)                  # [NT, P, 2048]
    arr = arr.reshape(NP, 2, P, H2).transpose(0, 2, 1, 3).reshape(NP, P, 2 * H2)
    return np.ascontiguousarray(arr).astype(ml_dtypes.bfloat16)


def _pack_pairs(a):
    """[BS, 1024] -> [NP, 128, 2048] with [q, p, 1024s+f] = a[256q+128s+p, f]."""
    return np.ascontiguousarray(
        a.reshape(NP, 2, P, HID).transpose(0, 2, 1, 3).reshape(NP, P, H2))


def _unpack_pairs(a):
    """inverse of _pack_pairs."""
    return np.ascontiguousarray(
        a.reshape(NP, P, 2, HID).transpose(0, 2, 1, 3).reshape(BS, HID))


def _run(x, h, weight, bias, trace=False, tmpdir=None):
    # wt[p, 192g+o] = W[g, o, p] — the exact SBUF layout, one contiguous DMA
    wt = np.ascontiguousarray(
        weight.transpose(2, 0, 1).reshape(P, G * OUT_PER)).astype(
        ml_dtypes.bfloat16)
    nc = _build_nc()
    in_maps = []
    for c in range(N_CORES):
        sl = slice(c * BS, (c + 1) * BS)
        xs, hs = x[sl], h[sl]
        in_maps.append({
            "hxt": _pack_hxt(hs, xs),
            "h2": _pack_pairs(hs),
            "wt": wt,
        })
    res = run_bass_kernel_spmd(nc, in_maps, core_ids=list(range(N_CORES)),
                               trace=trace, tmpdir=tmpdir)
    out = np.concatenate([_unpack_pairs(m["out"]) for m in res.results],
                         axis=0)
    return out, res


def kernel(x, h, weight, bias):
    x = np.asarray(x, dtype=np.float32)
    h = np.asarray(h, dtype=np.float32)
    weight = np.asarray(weight, dtype=np.float32)
    bias = np.asarray(bias, dtype=np.float32)
    if np.any(bias != 0.0):
        # setup_inputs() always passes zero bias; keep a correct fallback.
        return _np_reference(x, h, weight, bias)
    out, _ = _run(x, h, weight, bias)
    return out
